# revision 48
# baseline (speedup 1.0000x reference)
"""Trainium2 Bass kernel for nn_EncoderLayer (B=4, S=1024, D=1024, H=16, FF=2048).

Sharding: 8 cores = 4 batches x 2 sequence-halves. Each core redundantly
computes K/V projections for its whole batch (no collectives) and runs the
full layer for its own 512 query rows. Odd cores receive the sequence
rotated by 512 so local queries are always columns 0:512 (softmax over keys
is permutation-invariant, so K/V order doesn't matter).

On-chip layout is feature-major (activations stored transposed, [feature,
token]), which makes every matmul in the layer transpose-free:
  - weights are pre-tiled on the host into the exact [P, k, P] SBUF layout
    so every weight DMA reads contiguous 2KB-per-partition lines
  - per-feature biases / layernorm gamma,beta are per-partition scalars
  - softmax normalization (Z) comes free from a ones-augmented V column
  - layernorm stats are column sums computed with ones-vector matmuls

Every matmul is bf16 x bf16 (any fp32-family matmul disables FWL weight
loads program-wide, +20% on all matmuls). Precision is recovered off the
PE: f32 "hi-fi" shadows of q_proj/h1/f2 carry the residual + layernorm
paths on DVE, layernorm scale/bias rows stay f32 (gpsimd partition
broadcasts), and the output is stored f32. PSUM accumulation is f32.

DMA is split across both HWDGE rings (sync + scalar engines) because each
ring executes its DMAs serially in FIFO order.

Attention processes heads in pairs (even head in PE rows 0:63, odd head in
rows 64:127 -> concurrent matmuls via row tiling), with one [128,1024] exp
per (pair, key-tile) and softmax normalization deferred off the PE critical
path (fast approximate reciprocal + gpsimd partition broadcast).
"""

import sys
import types

import numpy as np
import ml_dtypes

BF16NP = ml_dtypes.bfloat16


def _shim_axon_hooks():
    """bass_utils imports antenv.axon_hooks in its trace path; the module is
    absent from this image. Provide a no-op stand-in (only used when tracing)."""
    try:
        import antenv.axon_hooks  # noqa: F401
    except Exception:
        mod = types.ModuleType("antenv.axon_hooks")
        mod.get_axon_ntff_profile_hook = lambda: None
        mod.set_axon_ntff_profile_hook = lambda h: None
        sys.modules["antenv.axon_hooks"] = mod


_shim_axon_hooks()

from concourse import bacc, mybir, tile  # noqa: E402
from concourse import bass_utils  # noqa: E402

F32 = mybir.dt.float32
BF16 = mybir.dt.bfloat16
AF = mybir.ActivationFunctionType

B, S, D, H, DH, FF = 4, 1024, 1024, 16, 64, 2048
SQ = 512          # query rows per core
P = 128
DT = D // P       # 8 d_model tiles
FT = FF // P      # 16 ffn tiles
ST = S // P       # 8 key tiles
NCORES = 8
EPS = 1e-6
SCALE = 1.0 / 32.0  # 1/sqrt(D_MODEL)

# consts layout (one [128, 64] f32 array, column ranges):
_C_BO = 0    # 8 cols: bo per d-tile
_C_B1 = 8    # 16 cols: b1 per f-tile
_C_B2 = 24   # 8 cols
_C_G1 = 32   # 8 cols
_C_BE1 = 40  # 8 cols
_C_G2 = 48   # 8 cols
_C_BE2 = 56  # 8 cols


def _emit(ctx, tc, aps, g1triv, g2triv, c2triv):
    nc = tc.nc
    (xT_ap, wq_ap, wk_ap, wv_ap, wo_ap, w1_ap, w2_ap, consts_ap, ones_ap,
     fold_ap, yT_ap) = aps

    acts = ctx.enter_context(tc.tile_pool(name="acts", bufs=1))
    wf = ctx.enter_context(tc.tile_pool(name="wf", bufs=7))
    sc2 = ctx.enter_context(tc.tile_pool(name="sc2", bufs=2))
    sc1 = ctx.enter_context(tc.tile_pool(name="sc1", bufs=1))
    pp = ctx.enter_context(tc.tile_pool(name="pp", bufs=2, space="PSUM"))
    pfill = ctx.enter_context(tc.tile_pool(name="pfill", bufs=1, space="PSUM"))
    pvp = ctx.enter_context(tc.tile_pool(name="pvp", bufs=3, space="PSUM"))

    def wslice(src_ap, row_tile, col_off=0, eng=None):
        """Load a [P, 8, P] stationary-weight tile from the host-pre-tiled
        weight layout: src[row_tile*P + p, col_off + k*P + n]. One contiguous
        2KB-per-partition DMA."""
        w = wf.tile([P, 8, P], BF16, tag="w", name="w")
        src = src_ap[row_tile * P:(row_tile + 1) * P,
                     col_off:col_off + 8 * P].rearrange("p (k n) -> p k n", n=P)
        (eng or nc.sync).dma_start(w[:], src)
        return w

    # ---- inputs (first Q weight pair ahead of x so MMs start ASAP;
    #      x tiles split across both HWDGE rings - each ring is a serial FIFO)
    ones_r = acts.tile([P, 1], BF16, tag="ones", name="ones")
    nc.sync.dma_start(ones_r[:], ones_ap[:])
    wq0 = wslice(wq_ap, 0)
    wq1 = wslice(wq_ap, 1, eng=nc.scalar)
    xt = []
    for j in range(DT):
        t = acts.tile([P, S], BF16, tag=f"xT{j}", name=f"xT{j}")
        (nc.sync if j % 2 == 0 else nc.scalar).dma_start(
            t[:], xT_ap[j * P:(j + 1) * P, :])
        xt.append(t)
    consts = acts.tile([P, 64], F32, tag="consts", name="consts")
    nc.scalar.dma_start(consts[:], consts_ap[:])
    fold2 = acts.tile([1, 2 * D], BF16, tag="fold", name="fold2")
    nc.scalar.dma_start(fold2[:], fold_ap[:])

    # warm-up: ~70 tiny matmuls keep the PE's HAM activity monitor busy
    # during the x/weight DMA window, so the real matmuls start at the full
    # 2.4 GHz clock instead of the 1.2 GHz cold gate
    wps = pvp.tile([1, 16], F32, tag="pv", name="warm")
    for _ in range(70):
        nc.tensor.matmul(wps[:, 0:1], ones_r[:, 0:1], ones_r[:, 0:1],
                         start=True, stop=True)

    # ---- Q projection (local 512 query columns), paired output columns ----
    qt, q_hi = [], []
    for j0 in range(0, DT, 2):
        wa = wq0 if j0 == 0 else wslice(wq_ap, j0)
        wb = wq1 if j0 == 0 else wslice(wq_ap, j0 + 1, eng=nc.scalar)
        ps = pp.tile([P, 2, SQ], F32, tag="ps2", name="ps2")
        for k in range(DT):
            nc.tensor.matmul(ps[:, 0, :], wa[:, k, :], xt[k][:, 0:SQ],
                             start=(k == 0), stop=(k == DT - 1))
            nc.tensor.matmul(ps[:, 1, :], wb[:, k, :], xt[k][:, 0:SQ],
                             start=(k == 0), stop=(k == DT - 1))
        for h in range(2):
            j = j0 + h
            q = acts.tile([P, SQ], BF16, tag=f"qT{j}", name=f"qT{j}")
            nc.scalar.copy(q[:], ps[:, h, :])
            qt.append(q)
            # f32 shadow of q_proj for the post-attention residual
            qh = acts.tile([P, SQ], F32, tag=f"qh{j}", name=f"qh{j}")
            nc.vector.tensor_copy(qh[:], ps[:, h, :])
            q_hi.append(qh)

    # ---- K projection groups (j=0..2 as a prefix; j=3..7 are emitted as
    #      fill quanta inside the attention loop, where the scalar engine is
    #      saturated by exp and the PE would otherwise idle) ----
    kt = []

    def k_group(j, w, copy_eng, pool):
        # two sequence-halves in sequence, each in a single-bank psum tile
        kj = acts.tile([P, S], BF16, tag=f"kT{j}", name=f"kT{j}")
        for h in (0, 1):
            if pool is pfill:
                psap = pfill.tile([P, SQ], F32, tag="fps", name="kps")[:]
            else:
                psap = pp.tile([P, 2, SQ], F32, tag="ps2", name="kps")[:, 0, :]
            for k in range(DT):
                nc.tensor.matmul(psap, w[:, k, :],
                                 xt[k][:, h * SQ:(h + 1) * SQ],
                                 start=(k == 0), stop=(k == DT - 1))
            copy_eng(kj[:, h * SQ:(h + 1) * SQ], psap)
        kt.append(kj)

    for j in range(3):
        k_group(j, wslice(wk_ap, j, eng=(nc.scalar if j % 2 else nc.sync)),
                nc.scalar.copy, pfill if j % 2 == 0 else pp)

    # ---- V projection, row-major with ones column: vr[st] = [P, H, DH+1] ----
    vr = []
    for st in range(ST):
        t = acts.tile([P, H, DH + 1], BF16, tag=f"vR{st}", name=f"vR{st}")
        nc.scalar.copy(t[:, :, DH:DH + 1], ones_r[:].to_broadcast((P, H, 1)))
        vr.append(t)

    def wv_loads(c, eng):
        wvk = []
        for k in range(DT):
            t = acts.tile([P, SQ], BF16, tag=f"wvc{k}", name=f"wvc{k}")
            eng.dma_start(t[:], wv_ap[k * P:(k + 1) * P, c * SQ:(c + 1) * SQ])
            wvk.append(t)
        return wvk

    def vr_group(c, st, wvk, copy_eng, pool):
        if pool is pfill:
            psap = pfill.tile([P, SQ], F32, tag="fps", name="vps")[:]
        else:
            psap = pp.tile([P, 2, SQ], F32, tag="ps2", name="vps")[:, 0, :]
        for k in range(DT):
            nc.tensor.matmul(psap, xt[k][:, st * P:(st + 1) * P],
                             wvk[k][:], start=(k == 0), stop=(k == DT - 1))
        copy_eng(vr[st][:, c * 8:(c + 1) * 8, 0:DH],
                 psap.rearrange("p (h d) -> p h d", d=DH))

    # c=0 half of V as a prefix (scalar-engine copies; exp hasn't started)
    wvk0 = wv_loads(0, nc.scalar)
    for st in range(ST):
        vr_group(0, st, wvk0, nc.scalar.copy, pfill if st % 2 == 0 else pp)

    # ---- fill quanta: single matmuls of K groups 3..7 and V c=1 groups,
    #      popped one-at-a-time between attention matmuls (DVE copies) ----
    wvk1 = wv_loads(1, nc.sync)
    kw = {j: wslice(wk_ap, j, eng=(nc.scalar if j % 2 else nc.sync))
          for j in range(3, DT)}
    fill = []

    def add_k_quanta(j):
        holder = {}

        def q(k, h):
            if k == 0:
                if h == 0:
                    holder['kj'] = acts.tile([P, S], BF16, tag=f"kT{j}",
                                             name=f"kT{j}")
                holder['ps'] = pfill.tile([P, SQ], F32, tag="fps", name="fps")[:]
            nc.tensor.matmul(holder['ps'], kw[j][:, k, :],
                             xt[k][:, h * SQ:(h + 1) * SQ],
                             start=(k == 0), stop=(k == DT - 1))
            if k == DT - 1:
                nc.vector.tensor_copy(
                    holder['kj'][:, h * SQ:(h + 1) * SQ], holder['ps'])
                if h == 1:
                    kt.append(holder['kj'])
        for h in (0, 1):
            for k in range(DT):
                fill.append(lambda k=k, h=h: q(k, h))

    def add_c1_quanta(st):
        holder = {}

        def q(k):
            if k == 0:
                holder['ps'] = pfill.tile([P, SQ], F32, tag="fps", name="fps")[:]
            nc.tensor.matmul(holder['ps'], xt[k][:, st * P:(st + 1) * P],
                             wvk1[k][:], start=(k == 0), stop=(k == DT - 1))
            if k == DT - 1:
                nc.vector.tensor_copy(
                    vr[st][:, 8:16, 0:DH],
                    holder['ps'].rearrange("p (h d) -> p h d", d=DH))
        for k in range(DT):
            fill.append(lambda k=k: q(k))

    add_k_quanta(3)
    for st in range(ST):
        add_c1_quanta(st)
    add_k_quanta(4)
    add_k_quanta(5)
    add_k_quanta(6)
    add_k_quanta(7)

    attn = [None] * DT

    def attn_pair(j, quota):
        pv0 = pvp.tile([DH + 1, SQ], F32, tag="pv", name="pv")
        pv1 = pvp.tile([DH + 1, SQ], F32, tag="pv", name="pv")
        prev = None
        for st in range(ST):
            sl = slice(st * P, (st + 1) * P)
            ps = pp.tile([P, 2, SQ], F32, tag="ps2", name="ps2")
            nc.tensor.matmul(ps[:, 0, :], kt[j][0:DH, sl], qt[j][0:DH, :],
                             start=True, stop=True)
            nc.tensor.matmul(ps[:, 1, :], kt[j][DH:P, sl], qt[j][DH:P, :],
                             start=True, stop=True)
            e2 = acts.tile([P, 2, SQ], BF16, tag=f"e{st % 4}", name="e2")
            nc.scalar.activation(e2[:], ps[:], AF.Exp, scale=SCALE)
            for _ in range(quota):
                if fill:
                    fill.pop(0)()
            # AV matmuls deferred one key-tile so the in-order PE queue never
            # parks on exp(st); the e-tile rotation (4 tags) keeps e2 alive
            if prev is not None:
                ep, stp = prev
                nc.tensor.matmul(pv0[:], vr[stp][:, 2 * j, :], ep[:, 0, :],
                                 start=(stp == 0), stop=False)
                nc.tensor.matmul(pv1[:], vr[stp][:, 2 * j + 1, :], ep[:, 1, :],
                                 start=(stp == 0), stop=False)
            prev = (e2, st)
        ep, stp = prev
        nc.tensor.matmul(pv0[:], vr[stp][:, 2 * j, :], ep[:, 0, :],
                         start=False, stop=True)
        nc.tensor.matmul(pv1[:], vr[stp][:, 2 * j + 1, :], ep[:, 1, :],
                         start=False, stop=True)
        attn[j] = acts.tile([P, SQ], BF16, tag=f"attnT{j}", name=f"attnT{j}")
        for half, pv in ((0, pv0), (1, pv1)):
            rows = slice(half * DH, half * DH + DH)
            zh = sc2.tile([1, SQ], F32, tag="zh", name="zh")
            nc.vector.tensor_copy(zh[:], pv[DH:DH + 1, :])
            iz = sc2.tile([1, SQ], F32, tag="zh", name="iz")
            nc.vector.reciprocal_approx_fast(iz[:], zh[:])
            bz = sc2.tile([DH, SQ], F32, tag="sb", name="sb")
            nc.gpsimd.partition_broadcast(bz[:], iz[:])
            nc.vector.tensor_mul(attn[j][rows, :], pv[0:DH, :], bz[:])

    for j in range(DT):
        attn_pair(j, quota=3 if j < 4 else 2)
    while fill:
        fill.pop(0)()

    # ---- output projection + relu + residual(q_proj) + LN1 ----
    # h_hi carries the f32 value for the layernorm/residual path; h1b is the
    # bf16 shadow the W1 matmuls and stats read.
    h_hi, h1b, sq1 = [], [], []
    for j0 in range(0, DT, 2):
        wa = wslice(wo_ap, j0)
        wb = wslice(wo_ap, j0 + 1, eng=nc.scalar)
        ps = pp.tile([P, 2, SQ], F32, tag="ps2", name="ps2")
        for k in range(DT):
            nc.tensor.matmul(ps[:, 0, :], wa[:, k, :], attn[k][:],
                             start=(k == 0), stop=(k == DT - 1))
            nc.tensor.matmul(ps[:, 1, :], wb[:, k, :], attn[k][:],
                             start=(k == 0), stop=(k == DT - 1))
        for h in range(2):
            j = j0 + h
            rel = sc2.tile([P, SQ], F32, tag="u", name="rel")
            nc.scalar.activation(rel[:], ps[:, h, :], AF.Relu,
                                 bias=consts[:, _C_BO + j:_C_BO + j + 1])
            t = acts.tile([P, SQ], F32, tag=f"h{j}", name=f"h1_{j}")
            nc.vector.tensor_add(t[:], rel[:], q_hi[j][:])
            h_hi.append(t)
            tb = acts.tile([P, SQ], BF16, tag=f"wvc{j}", name=f"h1b_{j}")
            nc.vector.tensor_copy(tb[:], t[:])
            h1b.append(tb)
            sq = acts.tile([P, SQ], BF16, tag=f"qT{j}", name=f"sq1_{j}")
            nc.vector.tensor_mul(sq[:], tb[:], tb[:])
            sq1.append(sq)
    # LN1 stats accumulate inside the W1 loop with a one-iteration lag (the
    # chain and broadcasts then run under W2's first block); gamma/beta are
    # folded into W1/W2 on the host, so W1 runs directly on h1b and the real
    # ln1 (residual only) is computed off the critical path during W1.
    ps_sum1 = pvp.tile([1, SQ], F32, tag="pv", name="ps_sum1")
    ps_sq1 = pvp.tile([1, SQ], F32, tag="pv", name="ps_sq1")

    def ln1_stats_step(j):
        nc.tensor.matmul(ps_sum1[:], ones_r[:], h1b[j][:],
                         start=(j == 0), stop=(j == DT - 1))
        nc.tensor.matmul(ps_sq1[:], ones_r[:], sq1[j][:],
                         start=(j == 0), stop=(j == DT - 1))

    def w1slice(f):
        return wslice(w1_ap, f, eng=(nc.scalar if f % 2 else nc.sync))

    hid = [None] * DT
    for f0 in range(0, FT, 2):
        wa = w1slice(f0)
        wb = w1slice(f0 + 1)
        ps = pp.tile([P, 2, SQ], F32, tag="ps2", name="ps2")
        for k in range(DT):
            nc.tensor.matmul(ps[:, 0, :], wa[:, k, :], h1b[k][:],
                             start=(k == 0), stop=(k == DT - 1))
            nc.tensor.matmul(ps[:, 1, :], wb[:, k, :], h1b[k][:],
                             start=(k == 0), stop=(k == DT - 1))
        for h in range(2):
            f = f0 + h
            m, half = f % DT, (f // DT) * SQ
            if hid[m] is None:
                hid[m] = acts.tile([P, S], BF16, tag=f"kT{m}", name=f"hid{m}")
            nc.scalar.copy(hid[m][:, half:half + SQ], ps[:, h, :])
        if f0 > 0:
            ln1_stats_step(f0 // 2 - 1)
    ln1_stats_step(DT - 1)

    s_sb = sc1.tile([1, SQ], F32, tag="s0", name="s_sb")
    nc.vector.tensor_copy(s_sb[:], ps_sum1[:])
    m2 = sc1.tile([1, SQ], F32, tag="s1", name="m2")
    nc.vector.tensor_mul(m2[:], s_sb[:], s_sb[:])
    a_t = sc1.tile([1, SQ], F32, tag="s2", name="a_t")
    nc.vector.scalar_tensor_tensor(a_t[:], m2[:], 1.0 / D, ps_sq1[:],
                                   op0=mybir.AluOpType.mult,
                                   op1=mybir.AluOpType.subtract)
    eps_t = sc1.tile([1, 1], F32, tag="eps", name="eps")
    nc.vector.memset(eps_t[:], EPS)
    sd1 = sc1.tile([1, SQ], F32, tag="s1", name="sd1")
    nc.scalar.activation(sd1[:], a_t[:], AF.Sqrt, bias=eps_t[:], scale=-1.0 / D)
    rstd1 = sc1.tile([1, SQ], F32, tag="s2", name="rstd1")
    nc.vector.reciprocal_approx_fast(rstd1[:], sd1[:])
    bneg1 = sc1.tile([1, SQ], F32, tag="s3", name="bneg1")
    nc.vector.scalar_tensor_tensor(bneg1[:], s_sb[:], -1.0 / D, rstd1[:],
                                   op0=mybir.AluOpType.mult,
                                   op1=mybir.AluOpType.mult)
    # bf16 rows for the W2-stage rank-1 fold matmuls
    negmu_r = sc1.tile([1, SQ], BF16, tag="s4", name="negmu_r")
    nc.vector.tensor_scalar_mul(negmu_r[:], s_sb[:], -1.0 / D)
    sd_r = sc1.tile([1, SQ], BF16, tag="s5", name="sd_r")
    nc.vector.tensor_copy(sd_r[:], sd1[:])
    # SBUF broadcasts of A=rstd and B=-mu*rstd (gpsimd; off critical path)
    abc_sb = sc2.tile([P, SQ], F32, tag="sb", name="abc_sb")
    nc.gpsimd.partition_broadcast(abc_sb[:], rstd1[:])
    bbc_sb = sc2.tile([P, SQ], F32, tag="zh", name="bbc_sb")
    nc.gpsimd.partition_broadcast(bbc_sb[:], bneg1[:])

    # real ln1 for the residual only: computed just-in-time per W2 iteration
    # (reads the SBUF broadcasts; keeps the DVE queue shallow so f2b[j] is
    # ready when the lagged LN2 stats matmul needs it)
    ln1 = []

    def ln1_tile(j):
        u = sc2.tile([P, SQ], F32, tag="u", name="u")
        nc.vector.tensor_mul(u[:], h_hi[j][:], abc_sb[:])
        d = acts.tile([P, SQ], F32, tag=f"ln{j}", name=f"ln1_{j}")
        if g1triv:
            # gamma1=1, beta1=0 (checked at build): the add IS the result
            nc.vector.tensor_add(d[:], u[:], bbc_sb[:])
        else:
            nc.vector.tensor_add(u[:], u[:], bbc_sb[:])
            nc.scalar.activation(d[:], u[:], AF.Identity,
                                 bias=consts[:, _C_BE1 + j:_C_BE1 + j + 1],
                                 scale=consts[:, _C_G1 + j:_C_G1 + j + 1])
        ln1.append(d)

    f2_hi, f2b, sq2 = [], [], []

    def w2slice(j, half):
        w = acts.tile([P, 8, P], BF16, tag=f"xT{(2 * j + half) % 8}", name="w2t")
        src = w2_ap[j * P:(j + 1) * P, half * 8 * P:(half + 1) * 8 * P]
        (nc.sync if half == 0 else nc.scalar).dma_start(
            w[:], src.rearrange("p (k n) -> p k n", n=P))
        return w

    # LN2 stats accumulate inside the W2 loop with a one-iteration lag so the
    # in-order PE queue never waits on the DVE squares.
    ps_sum2 = pvp.tile([1, SQ], F32, tag="pv", name="ps_sum2")
    ps_sq2 = pvp.tile([1, SQ], F32, tag="pv", name="ps_sq2")

    def stats_step(j):
        nc.tensor.matmul(ps_sum2[:], ones_r[:], f2b[j][:],
                         start=(j == 0), stop=(j == DT - 1))
        nc.tensor.matmul(ps_sq2[:], ones_r[:], sq2[j][:],
                         start=(j == 0), stop=(j == DT - 1))

    for j in range(DT):
        wa = w2slice(j, 0)
        wb = w2slice(j, 1)
        ln1_tile(j)
        ps = pp.tile([P, 2, SQ], F32, tag="ps2", name="ps2")
        for f in range(FT):
            w = wa if f < 8 else wb
            m, half = f % DT, (f // DT) * SQ
            nc.tensor.matmul(ps[:, 0, :], w[:, f % 8, :],
                             hid[m][:, half:half + SQ],
                             start=(f == 0), stop=False)
        # rank-1 corrections: + (-mu) x w2g1[d]  + sd x c2[d] (the sd term
        # drops out when c2 == 0, checked at build)
        nc.tensor.matmul(ps[:, 0, :], fold2[0:1, j * P:(j + 1) * P],
                         negmu_r[:], start=False, stop=c2triv)
        if not c2triv:
            nc.tensor.matmul(ps[:, 0, :], fold2[0:1, D + j * P:D + (j + 1) * P],
                             sd_r[:], start=False, stop=True)
        # ff_pre = A * psum ; relu(A*x) = A*relu(x) since A=rstd>0
        rel = sc2.tile([P, SQ], F32, tag="u", name="rel2")
        nc.vector.scalar_tensor_tensor(rel[:], ps[:, 0, :], 0.0, abc_sb[:],
                                       op0=mybir.AluOpType.max,
                                       op1=mybir.AluOpType.mult)
        t = acts.tile([P, SQ], F32, tag=f"qh{j}", name=f"f2_{j}")
        nc.vector.tensor_add(t[:], rel[:], ln1[j][:])
        f2_hi.append(t)
        tb = acts.tile([P, SQ], BF16, tag=f"qT{j}", name=f"f2b_{j}")
        nc.vector.tensor_copy(tb[:], t[:])
        f2b.append(tb)
        sq = acts.tile([P, SQ], BF16, tag=f"wvc{j}", name=f"sq2_{j}")
        nc.vector.tensor_mul(sq[:], tb[:], tb[:])
        sq2.append(sq)
        if j > 0:
            stats_step(j - 1)
    stats_step(DT - 1)

    # ---- LN2 chain + apply + store ----
    s_sb = sc1.tile([1, SQ], F32, tag="s0", name="s_sb2")
    nc.vector.tensor_copy(s_sb[:], ps_sum2[:])
    m2 = sc1.tile([1, SQ], F32, tag="s1", name="m2b")
    nc.vector.tensor_mul(m2[:], s_sb[:], s_sb[:])
    a_t = sc1.tile([1, SQ], F32, tag="s2", name="a_t2")
    nc.vector.scalar_tensor_tensor(a_t[:], m2[:], 1.0 / D, ps_sq2[:],
                                   op0=mybir.AluOpType.mult,
                                   op1=mybir.AluOpType.subtract)
    eps_t = sc1.tile([1, 1], F32, tag="eps", name="eps2")
    nc.vector.memset(eps_t[:], EPS)
    sd2 = sc1.tile([1, SQ], F32, tag="s1", name="sd2")
    nc.scalar.activation(sd2[:], a_t[:], AF.Sqrt, bias=eps_t[:], scale=-1.0 / D)
    rstd2 = sc1.tile([1, SQ], F32, tag="s2", name="rstd2")
    nc.vector.reciprocal_approx_fast(rstd2[:], sd2[:])
    # bf16 apply path: the bf16 rounding of f2/rstd rows costs ~3e-3 of the
    # 2e-2 error budget; bneg2 is produced in bf16 straight from the STT
    b_rb = sc1.tile([1, SQ], BF16, tag="s7", name="b_rb")
    nc.vector.scalar_tensor_tensor(b_rb[:], ps_sum2[:], -1.0 / D, rstd2[:],
                                   op0=mybir.AluOpType.mult,
                                   op1=mybir.AluOpType.mult)
    a_rb = sc1.tile([1, SQ], BF16, tag="s6", name="a_rb")
    nc.vector.tensor_copy(a_rb[:], rstd2[:])
    a_sb = sc2.tile([P, SQ], BF16, tag="sb", name="a_sb")
    nc.gpsimd.partition_broadcast(a_sb[:], a_rb[:])
    b_sb = sc2.tile([P, SQ], BF16, tag="zh", name="b_sb")
    nc.gpsimd.partition_broadcast(b_sb[:], b_rb[:])
    for j in range(DT):
        u = sc2.tile([P, SQ], BF16, tag="u", name="u2")
        nc.vector.tensor_mul(u[:], f2b[j][:], a_sb[:])
        d = acts.tile([P, SQ], BF16, tag=f"yb{j}", name=f"y_{j}")
        if g2triv:
            # gamma2=1, beta2=0 (checked at build): skip the identity pass
            nc.vector.tensor_add(d[:], u[:], b_sb[:])
        else:
            nc.vector.tensor_add(u[:], u[:], b_sb[:])
            nc.scalar.activation(d[:], u[:], AF.Identity,
                                 bias=consts[:, _C_BE2 + j:_C_BE2 + j + 1],
                                 scale=consts[:, _C_G2 + j:_C_G2 + j + 1])
        (nc.sync if j % 2 == 0 else nc.scalar).dma_start(
            yT_ap[j * P:(j + 1) * P, :], d[:])


def build(g1triv=False, g2triv=False, c2triv=False):
    nc = bacc.Bacc("TRN2", target_bir_lowering=False, debug=False,
                   num_devices=NCORES)
    xT_ap = nc.dram_tensor("xT", [D, S], BF16, kind="ExternalInput").ap()
    wq_ap = nc.dram_tensor("WqT", [D, D], BF16, kind="ExternalInput").ap()
    wk_ap = nc.dram_tensor("WkT", [D, D], BF16, kind="ExternalInput").ap()
    wv_ap = nc.dram_tensor("Wv", [D, D], BF16, kind="ExternalInput").ap()
    wo_ap = nc.dram_tensor("WoT", [D, D], BF16, kind="ExternalInput").ap()
    w1_ap = nc.dram_tensor("W1T", [FF, D], BF16, kind="ExternalInput").ap()
    w2_ap = nc.dram_tensor("W2T", [D, FF], BF16, kind="ExternalInput").ap()
    consts_ap = nc.dram_tensor("consts", [P, 64], F32, kind="ExternalInput").ap()
    ones_ap = nc.dram_tensor("ones", [P, 1], BF16, kind="ExternalInput").ap()
    fold_ap = nc.dram_tensor("fold2", [1, 2 * D], BF16, kind="ExternalInput").ap()
    yT_ap = nc.dram_tensor("yT", [D, SQ], BF16, kind="ExternalOutput").ap()
    aps = (xT_ap, wq_ap, wk_ap, wv_ap, wo_ap, w1_ap, w2_ap, consts_ap,
           ones_ap, fold_ap, yT_ap)
    from contextlib import ExitStack
    with tile.TileContext(nc) as tc, ExitStack() as ctx:
        _emit(ctx, tc, aps, g1triv, g2triv, c2triv)
    nc.compile()
    return nc


_cached_nc = {}


def _get_nc(g1triv=False, g2triv=False, c2triv=False):
    key = (g1triv, g2triv, c2triv)
    if key not in _cached_nc:
        _cached_nc[key] = build(g1triv, g2triv, c2triv)
    return _cached_nc[key]


def _tile_proj(W):
    """[1024,1024] -> tiled so row j*128+p, line (k,n) is W[k*128+p, j*128+n]."""
    return np.ascontiguousarray(
        W.reshape(8, P, 8, P).transpose(2, 1, 0, 3).reshape(D, D))


def _prep_in_maps(x, Wq, Wk, Wv, Wo, bo, ln1_g, ln1_b, W1, b1, W2, b2,
                  ln2_g, ln2_b):
    f = np.float32
    consts = np.zeros((P, 64), f)
    consts[:, _C_BO:_C_BO + 8] = np.asarray(bo, f).reshape(8, P).T
    consts[:, _C_B1:_C_B1 + 16] = np.asarray(b1, f).reshape(16, P).T
    consts[:, _C_B2:_C_B2 + 8] = np.asarray(b2, f).reshape(8, P).T
    consts[:, _C_G1:_C_G1 + 8] = np.asarray(ln1_g, f).reshape(8, P).T
    consts[:, _C_BE1:_C_BE1 + 8] = np.asarray(ln1_b, f).reshape(8, P).T
    consts[:, _C_G2:_C_G2 + 8] = np.asarray(ln2_g, f).reshape(8, P).T
    consts[:, _C_BE2:_C_BE2 + 8] = np.asarray(ln2_b, f).reshape(8, P).T
    ones = np.ones((P, 1), BF16NP)
    W1f = np.asarray(W1, np.float64)
    W2f = np.asarray(W2, np.float64)
    g1v = np.asarray(ln1_g, np.float64)
    b1v = np.asarray(ln1_b, np.float64)
    g1 = (g1v[:, None] * W1f).sum(axis=0)            # [FF]
    c1 = np.asarray(b1, np.float64) + (b1v[:, None] * W1f).sum(axis=0)
    w2g1 = g1 @ W2f                                   # [D]
    c2 = np.asarray(b2, np.float64) + c1 @ W2f        # [D]
    fold2 = np.concatenate([w2g1, c2]).astype(f)[None, :]
    W1g = (g1v[:, None] * W1f).astype(f)
    # host pre-tiling into contiguous [P, k, P] DMA layout
    W1T = np.ascontiguousarray(
        W1g.reshape(8, P, 16, P).transpose(2, 1, 0, 3).reshape(FF, D)
    ).astype(BF16NP)
    W2T = np.ascontiguousarray(
        np.asarray(W2, f).reshape(16, P, 8, P).transpose(2, 1, 0, 3)
        .reshape(D, FF)).astype(BF16NP)
    shared = {
        "WqT": _tile_proj(np.asarray(Wq, f)).astype(BF16NP),
        "WkT": _tile_proj(np.asarray(Wk, f)).astype(BF16NP),
        "Wv": np.ascontiguousarray(Wv, f).astype(BF16NP),
        "WoT": _tile_proj(np.asarray(Wo, f)).astype(BF16NP),
        "W1T": W1T, "W2T": W2T,
        "consts": consts, "ones": ones, "fold2": fold2.astype(BF16NP),
    }
    xt = np.asarray(x, f).transpose(0, 2, 1)  # [B, D, S]
    in_maps = []
    for core in range(NCORES):
        b, off = core // 2, (core % 2) * SQ
        if off == 0:
            xrot = xt[b]
        else:
            # rotate so this core's query rows are columns 0:SQ; key order is
            # irrelevant (softmax sums over all keys)
            xrot = np.concatenate([xt[b][:, off:], xt[b][:, :off]], axis=1)
        in_maps.append(dict(shared, xT=np.ascontiguousarray(xrot).astype(BF16NP)))
    return in_maps


def run(inputs, trace=False, tmpdir=None):
    """Run the kernel on 8 cores. Returns (y, BassKernelResults)."""
    # specialize the build when the layernorm affines are trivial (skips the
    # per-tile identity pass; the general path remains for other inputs)
    g1triv = bool(np.allclose(np.asarray(inputs["ln1_g"], np.float32), 1.0)
                  and np.allclose(np.asarray(inputs["ln1_b"], np.float32), 0.0))
    g2triv = bool(np.allclose(np.asarray(inputs["ln2_g"], np.float32), 1.0)
                  and np.allclose(np.asarray(inputs["ln2_b"], np.float32), 0.0))
    c2 = (np.asarray(inputs["b2"], np.float64)
          + (np.asarray(inputs["b1"], np.float64)
             + (np.asarray(inputs["ln1_b"], np.float64)[:, None]
                * np.asarray(inputs["W1"], np.float64)).sum(axis=0))
          @ np.asarray(inputs["W2"], np.float64))
    c2triv = bool(np.max(np.abs(c2)) == 0.0)
    nc = _get_nc(g1triv, g2triv, c2triv)
    in_maps = _prep_in_maps(
        inputs["x"], inputs["Wq"], inputs["Wk"], inputs["Wv"], inputs["Wo"],
        inputs["bo"], inputs["ln1_g"], inputs["ln1_b"], inputs["W1"],
        inputs["b1"], inputs["W2"], inputs["b2"], inputs["ln2_g"],
        inputs["ln2_b"])
    try:
        res = bass_utils.run_bass_kernel_spmd(nc, in_maps, list(range(NCORES)),
                                              trace=trace, tmpdir=tmpdir)
    except Exception:
        # transient NRT wedge right after NEFF load; retry once on a clean run
        import time as _time
        _time.sleep(2.0)
        res = bass_utils.run_bass_kernel_spmd(nc, in_maps, list(range(NCORES)),
                                              trace=trace, tmpdir=tmpdir)
    y = np.empty((B, S, D), np.float32)
    for core in range(NCORES):
        b, off = core // 2, (core % 2) * SQ
        y[b, off:off + SQ, :] = res.results[core]["yT"].T.astype(np.float32)
    return y, res


def kernel(x, mask, Wq, Wk, Wv, Wo, bo, ln1_g, ln1_b, W1, b1, W2, b2,
           ln2_g, ln2_b):
    # mask is all-ones per the problem spec (fill: ones) -> identity in the
    # reference's jnp.where; accepted but unused.
    y, _ = run(dict(x=x, Wq=Wq, Wk=Wk, Wv=Wv, Wo=Wo, bo=bo, ln1_g=ln1_g,
                    ln1_b=ln1_b, W1=W1, b1=b1, W2=W2, b2=b2, ln2_g=ln2_g,
                    ln2_b=ln2_b))
    return y
